# revision 12
# baseline (speedup 1.0000x reference)
"""GAT message-passing kernel for TRN2 (8-core SPMD).

Math (heads h, nodes n):
  t[n,h,:] = x[n] @ Ws[h].T            (t-space features, 64 per head)
  Ar[n,h]  = x[n] @ war[:,h]           (war = Ws[h].T @ a_r[h], folded weights)
  u        = exp(Ar)
  out[i, h*64:h*64+64] = elu( sum_{e:src=i} u[dst,h]*t[dst,h,:] / sum u[dst,h] )

Sharding: src-range per core. Phase 1 builds the Y table
[u*t (512) | u (8) | pad], row stride 640 cols bf16 (1280B), for all nodes
(replicated on every core, in core-private DRAM, split in two halves so
gather indices fit int16). Phase 2 gathers 528 of the 640 cols per edge
(row reads need not be 256B-multiples; only the row STRIDE does), builds a
one-hot S on DVE, and segment-sums via PE matmul into PSUM per 128-node
window (N=512 numerator + N=8 denominator); then normalize + elu + store.
Per-(window,half) slot counts are exact (16-mult) with trailing -1 indices
that the gather ucode skips, so padding transfers almost nothing.
"""

import math
import numpy as np
from contextlib import ExitStack

import concourse.bass as bass
import concourse.bacc as bacc
import concourse.mybir as mybir
import concourse.ap_utils as ap_utils
from concourse.tile import TileContext
from concourse.tile import add_dep_helper

F32 = mybir.dt.float32
BF16 = mybir.dt.bfloat16
FP8 = mybir.dt.float8e4
I16 = mybir.dt.int16

P = 128
IN_FEAT = 256
HEADS = 8
OUT = 64
TD = HEADS * OUT  # 512
YW = 640          # row stride in cols (1280B, mult of 256B)
GELEM = 528       # gathered cols per row (1056B; 520 used + 16B junk)
KSLAB = 8         # x tiles per input DMA
KYW = 4           # tiles per y-write DMA


def dma_gather_raw(gp, out_ap, in_ap, idxs_ap, num_idxs, elem_size, elem_step,
                   single_packet=True, queue_num=0):
    """BassGpSimd.dma_gather clone without the elem_size%256 restriction
    (that restriction only applies to transpose mode; row reads of any size
    work as long as the row stride is a 256B multiple)."""
    from concourse.bass import MemorySpace, exact_div, round_up_to_multiple
    gp._assert_queue_num(queue_num)
    assert idxs_ap.dtype == mybir.dt.int16
    assert in_ap.dtype == out_ap.dtype
    assert in_ap.space == MemorySpace.DRAM
    assert idxs_ap.space == MemorySpace.SBUF
    assert out_ap.space == MemorySpace.SBUF
    assert ap_utils.ap_is_contiguous(out_ap.ap[1:])
    assert ap_utils.ap_is_contiguous(idxs_ap.ap[1:])
    assert in_ap.ap[-1][1] == out_ap.ap[-1][1] == elem_size
    assert out_ap.ap[0][1] * out_ap.ap[1][1] >= round_up_to_multiple(num_idxs, 128)
    assert in_ap.ap[0][0] == elem_step
    stride_bytes_256 = exact_div(elem_step * mybir.dt.size(in_ap.dtype), 256)
    assert stride_bytes_256 < 256
    _in_ap = gp.lower_ap_dma(in_ap, for_custom_bir_dma=True)
    inst = gp.add_instruction(
        mybir.InstDMAGatherAnt(
            name=gp.bass.get_next_instruction_name(),
            ins=[*_in_ap, gp.lower_ap(idxs_ap),
                 gp.lower_val_access(gp.to_reg(num_idxs))],
            outs=[gp.lower_ap(out_ap)],
            transpose=False,
            num_idxs=num_idxs,
            elem_size=elem_size,
            stride_bytes_256=stride_bytes_256,
            gen_mode=0,
            single_packet=single_packet,
            queue_num=queue_num,
            sbuf_tokens_per_rank=0,
            sbuf_free_dim_per_rank=0,
            sbuf_free_dim_pad_per_rank=0,
            sbuf_byte_offset=0,
        )
    )
    return inst


class Config:
    def __init__(self, n_nodes, src, dst, n_cores=8, ch_max=9):
        self.n_cores = n_cores
        self.n_nodes = n_nodes
        self.w_per_core = math.ceil(n_nodes / (n_cores * P))
        self.npc = self.w_per_core * P
        self.n_pad = self.npc * n_cores
        self.x_tiles = self.n_pad // P
        h0_tiles = self.x_tiles // 2
        assert h0_tiles * P < 32768 and (self.x_tiles - h0_tiles) * P < 32768
        assert h0_tiles % KYW == 0 and self.x_tiles % KSLAB == 0
        self.h0_tiles = h0_tiles
        self.h0_rows = h0_tiles * P
        self.h1_rows = (self.x_tiles - h0_tiles) * P
        self.n_slabs = self.x_tiles // KSLAB
        self.ch_max = ch_max

        W = self.w_per_core
        src = np.asarray(src, dtype=np.int64)
        dst = np.asarray(dst, dtype=np.int64)
        core = src // self.npc
        w = (src % self.npc) // P
        lsrc = src % P
        half = (dst >= self.h0_rows).astype(np.int64)
        lidx = dst - self.h0_rows * half

        counts = np.zeros((n_cores, W, 2), dtype=np.int64)
        np.add.at(counts, (core, w, half), 1)
        cap = counts.max(axis=0)  # [W, 2]
        nblk = np.maximum(np.ceil(cap / P).astype(np.int64), 1)  # blocks per (w, half)
        self.nblk = nblk
        self.tot_blocks = int(nblk.sum())
        self.tot_idx = self.tot_blocks * P

        # block offsets per (w, half) in the packed stream (same per core)
        blk_off = np.zeros((W, 2), dtype=np.int64)
        acc = 0
        for wi in range(W):
            for hi in range(2):
                blk_off[wi, hi] = acc
                acc += nblk[wi, hi]

        # order edges by (core, w, half); rank within group -> slot
        key = (core * W + w) * 2 + half
        order = np.argsort(key, kind="stable")
        s_core, s_w, s_half = core[order], w[order], half[order]
        s_lsrc, s_lidx = lsrc[order], lidx[order]
        gkey = (s_core * W + s_w) * 2 + s_half
        change = np.r_[True, gkey[1:] != gkey[:-1]]
        grp_start = np.flatnonzero(change)
        grp_id = np.cumsum(change) - 1
        grp_rank = np.arange(len(order)) - grp_start[grp_id]
        slot = blk_off[s_w, s_half] * P + grp_rank

        # call table: (w, half, b0 global blocks, nb, num_idxs); full blocks
        # (every slot of every block is gathered -> no stale/uninit SBUF)
        calls = []
        for wi in range(W):
            for hi in range(2):
                nb_all = int(nblk[wi, hi])
                b0 = int(blk_off[wi, hi])
                off = 0
                while off < nb_all:
                    nb = min(ch_max, nb_all - off)
                    calls.append((wi, hi, b0 + off, nb, nb * P))
                    off += nb
        self.calls = calls

        # idx wrapped [16, tot_idx/16] replicated to 128 partitions.
        # slot i of a call starting at slot g0 (mult of 128) -> [i%16, g0/16+i/16]
        # (global slots work directly since calls slice the same stream)
        # Pad slots gather row 0 (harmless; meta=-1 makes S zero there).
        idx16 = np.zeros((n_cores, 16, self.tot_idx // 16), np.int16)
        row16 = slot % 16
        col16 = slot // 16
        idx16[s_core, row16, col16] = s_lidx.astype(np.int16)
        self.idx_packed = np.tile(idx16, (1, 8, 1))

        self.meta_packed = np.full((n_cores, P, self.tot_blocks), -1.0, np.float32)
        blk = slot // P
        pslot = slot % P
        self.meta_packed[s_core, pslot, blk] = s_lsrc.astype(np.float32)

        self.pad_frac = (self.tot_idx * n_cores) / max(1, len(src)) - 1.0


def build_program(cfg: Config):
    nc = bacc.Bacc("TRN2", target_bir_lowering=False, debug=False,
                   num_devices=cfg.n_cores, num_swdge_queues=4)
    W = cfg.w_per_core

    xt_d = nc.dram_tensor("xt", [P, cfg.n_slabs, 2, KSLAB * P], BF16,
                          kind="ExternalInput")
    wcat_d = nc.dram_tensor("wcat", [IN_FEAT, TD], BF16, kind="ExternalInput")
    war_d = nc.dram_tensor("war", [IN_FEAT, HEADS], BF16, kind="ExternalInput")
    iota_d = nc.dram_tensor("iota", [P, P], BF16, kind="ExternalInput")
    idx_d = nc.dram_tensor("idx", [128, cfg.tot_idx // 16], I16,
                           kind="ExternalInput")
    meta_d = nc.dram_tensor("meta", [P, cfg.tot_blocks], BF16,
                            kind="ExternalInput")
    out_d = nc.dram_tensor("out", [cfg.npc, TD], F32, kind="ExternalOutput")
    y0_d = nc.dram_tensor("y0", [cfg.h0_rows, YW], BF16, kind="Internal")
    y1_d = nc.dram_tensor("y1", [cfg.h1_rows, YW], BF16, kind="Internal")

    y_writes = [[], []]  # per half
    with TileContext(nc) as tc:
        with ExitStack() as ctx:
            # ---- constants for BOTH phases, loaded up front so phase-2
            # gathers are not queued behind phase-1 DMA traffic ----
            consts = ctx.enter_context(tc.tile_pool(name="consts", bufs=1))
            wc = consts.tile([P, 2, TD], BF16, tag="wc")
            nc.sync.dma_start(wc[:, :, :], wcat_d.rearrange("(c p) n -> p c n", p=P))
            wr = consts.tile([P, 2, HEADS], BF16, tag="wr")
            nc.sync.dma_start(wr[:, :, :], war_d.rearrange("(c p) n -> p c n", p=P))
            iota = consts.tile([P, P], BF16, tag="iota")
            nc.sync.dma_start(iota[:, :], iota_d[:, :])
            idx_sb = consts.tile([128, cfg.tot_idx // 16], I16, tag="idx")
            nc.sync.dma_start(idx_sb[:, :], idx_d[:, :])
            meta_sb = consts.tile([P, cfg.tot_blocks], BF16, tag="meta")
            nc.sync.dma_start(meta_sb[:, :], meta_d[:, :])
            neg1 = consts.tile([P, 1], F32, tag="neg1")
            nc.vector.memset(neg1[:, :], -1.0)

            # ---------------- phase 1: build Y table ----------------
            with ExitStack() as p1:
                xin = p1.enter_context(tc.tile_pool(name="xin", bufs=3))
                tcp = p1.enter_context(tc.tile_pool(name="tcp", bufs=3))
                yout = p1.enter_context(tc.tile_pool(name="yout", bufs=4))
                ps_t = p1.enter_context(tc.tile_pool(name="ps_t", bufs=2,
                                                     space="PSUM"))
                ysb = None
                pt2 = None
                for t in range(cfg.x_tiles):
                    s, k = divmod(t, KSLAB)
                    if k == 0:
                        xT = xin.tile([P, 2, KSLAB * P], BF16)
                        nc.sync.dma_start(xT[:, :, :], xt_d[:, s, :, :])
                    if t % KYW == 0:
                        ysb = yout.tile([P, KYW, 520], BF16)
                    k2 = t % KYW
                    lhs0 = xT[:, 0, k * P:(k + 1) * P]
                    lhs1 = xT[:, 1, k * P:(k + 1) * P]
                    # pt for 2 tiles in one 2-bank PSUM tile so the scalar
                    # PSUM->SBUF copy runs once per 2 tiles
                    hf = t % 2
                    if hf == 0:
                        pt2 = ps_t.tile([P, 2, TD], F32, tag="pt")
                    par = ps_t.tile([P, HEADS], F32, tag="par")
                    nc.tensor.matmul(par[:, :], lhs0, wr[:, 0, :], start=True, stop=False)
                    nc.tensor.matmul(par[:, :], lhs1, wr[:, 1, :], start=False, stop=True)
                    nc.tensor.matmul(pt2[:, hf, :], lhs0, wc[:, 0, :], start=True,
                                     stop=False, skip_group_check=True)
                    nc.tensor.matmul(pt2[:, hf, :], lhs1, wc[:, 1, :], start=False,
                                     stop=True, skip_group_check=True)
                    # u = exp(Ar) -> row cols 512:520 (bf16)
                    nc.scalar.activation(ysb[:, k2, TD:520], par[:, :],
                                         mybir.ActivationFunctionType.Exp)
                    if hf == 1:
                        tcb = tcp.tile([P, 2, TD], BF16)
                        nc.scalar.activation(tcb[:, :, :], pt2[:, :, :],
                                             mybir.ActivationFunctionType.Identity)
                        # t~ = t*u for both tiles in one DVE op
                        nc.vector.tensor_tensor(
                            ysb[:, k2 - 1:k2 + 1, 0:TD].rearrange(
                                "p k (h o) -> p k h o", h=HEADS),
                            tcb[:, :, :].rearrange("p k (h o) -> p k h o", h=HEADS),
                            ysb[:, k2 - 1:k2 + 1, TD:520].unsqueeze(3)
                                .broadcast_to([P, 2, HEADS, OUT]),
                            mybir.AluOpType.mult,
                        )
                    if k2 == KYW - 1:
                        g4 = t // KYW
                        if t < cfg.h0_tiles:
                            dst = y0_d[(g4 * KYW) * P:(g4 * KYW + KYW) * P, 0:520]
                        else:
                            tt = g4 - cfg.h0_tiles // KYW
                            dst = y1_d[(tt * KYW) * P:(tt * KYW + KYW) * P, 0:520]
                        wi_ = nc.sync.dma_start(
                            dst.rearrange("(k p) c -> p k c", p=P), ysb[:, :, :])
                        y_writes[0 if t < cfg.h0_tiles else 1].append(wi_)

            # ---------------- phase 2: gather + segment sums ----------------
            gpool = ctx.enter_context(tc.tile_pool(name="gath", bufs=8))
            spool = ctx.enter_context(tc.tile_pool(name="onehot", bufs=4))
            opool = ctx.enter_context(tc.tile_pool(name="outp", bufs=3))
            ps_num = ctx.enter_context(tc.tile_pool(name="ps_num", bufs=3,
                                                    space="PSUM"))
            ps_den = ctx.enter_context(tc.tile_pool(name="ps_den", bufs=3,
                                                    space="PSUM"))

            fence_pending = [True, True]
            qn = [0]

            calls_by_w = [[] for _ in range(W)]
            for (wi, hi, b0, nb, nidx) in cfg.calls:
                calls_by_w[wi].append((hi, b0, nb, nidx))

            for wi in range(W):
                wcalls = calls_by_w[wi]
                nblk_w = sum(nb for (_, _, nb, _) in wcalls)
                pn = ps_num.tile([P, TD], F32, tag="pn")
                pd = ps_den.tile([P, HEADS], F32, tag="pd")
                bi = 0
                for (hi, b0, nb, nidx) in wcalls:
                    g = gpool.tile([P, cfg.ch_max, GELEM], BF16)
                    src_t = y0_d if hi == 0 else y1_d
                    g_inst = dma_gather_raw(
                        nc.gpsimd,
                        out_ap=g[:, 0:nb, :],
                        in_ap=src_t[:, 0:GELEM],
                        idxs_ap=idx_sb[:, b0 * 8:b0 * 8 + (nidx + 15) // 16],
                        num_idxs=nidx,
                        elem_size=GELEM,
                        elem_step=YW,
                        single_packet=(nidx <= 1024),
                        queue_num=qn[0],
                    )
                    qn[0] = (qn[0] + 1) % 4
                    if fence_pending[hi]:
                        # gather's indexed DRAM read of Y is invisible to
                        # Tile; gathers run in order on GpSimd, so gating
                        # the first gather per half on that half's writes
                        # fences all of them.
                        for wr_ in y_writes[hi]:
                            add_dep_helper(g_inst.ins, wr_.ins,
                                           reason="gather reads Y table")
                        fence_pending[hi] = False
                    s = spool.tile([P, cfg.ch_max, P], FP8)
                    nc.vector.tensor_tensor(
                        s[:, 0:nb, :],
                        meta_sb[:, b0:b0 + nb].unsqueeze(2).broadcast_to([P, nb, P]),
                        iota[:, :].unsqueeze(1).broadcast_to([P, nb, P]),
                        mybir.AluOpType.is_equal,
                    )
                    for j in range(nb):
                        st = (bi == 0)
                        sp = (bi == nblk_w - 1)
                        nc.tensor.matmul(pn[:, :], s[:, j, :], g[:, j, 0:TD],
                                         start=st, stop=sp, skip_group_check=True)
                        nc.tensor.matmul(pd[:, :], s[:, j, :], g[:, j, TD:520],
                                         start=st, stop=sp, skip_group_check=True)
                        bi += 1
                # ---- evict window ----
                den = opool.tile([P, HEADS], F32, tag="den")
                nc.vector.tensor_scalar_add(den[:, :], pd[:, :], 1e-30)
                rden = opool.tile([P, HEADS], F32, tag="rden")
                nc.vector.reciprocal(rden[:, :], den[:, :])
                hout = opool.tile([P, TD], F32, tag="hout")
                nc.vector.tensor_tensor(
                    hout[:, :].rearrange("p (h o) -> p h o", h=HEADS),
                    pn[:, :].rearrange("p (h o) -> p h o", h=HEADS),
                    rden[:, :].unsqueeze(2).broadcast_to([P, HEADS, OUT]),
                    mybir.AluOpType.mult,
                )
                # elu(z) = max(z,0) + exp(min(z,0)) - 1; min(z,0) = -relu(-z)
                xm = opool.tile([P, TD], F32, tag="xm")
                nc.scalar.activation(xm[:, :], hout[:, :],
                                     mybir.ActivationFunctionType.Relu,
                                     scale=-1.0)
                ex = opool.tile([P, TD], F32, tag="ex")
                nc.scalar.activation(ex[:, :], xm[:, :],
                                     mybir.ActivationFunctionType.Exp,
                                     scale=-1.0)
                fin = opool.tile([P, TD], F32, tag="fin")
                nc.vector.scalar_tensor_tensor(
                    out=fin[:, :], in0=hout[:, :], scalar=0.0, in1=ex[:, :],
                    op0=mybir.AluOpType.max, op1=mybir.AluOpType.add,
                )
                fin2 = opool.tile([P, TD], F32, tag="fin2")
                nc.scalar.activation(fin2[:, :], fin[:, :],
                                     mybir.ActivationFunctionType.Identity,
                                     bias=neg1[:, :])
                nc.sync.dma_start(out_d[wi * P:(wi + 1) * P, :], fin2[:, :])

    nc.compile()
    return nc


def host_prep(cfg: Config, x, Ws, As):
    import ml_dtypes as _md
    x = np.asarray(x, np.float32)
    Ws = np.asarray(Ws, np.float32)
    As = np.asarray(As, np.float32)
    n = x.shape[0]
    xp = np.zeros((cfg.n_pad, IN_FEAT), np.float32)
    xp[:n] = x
    # xt[p, s, c, j] = x[s*KSLAB*P + j, c*128 + p]
    xt = (xp.reshape(cfg.n_slabs, KSLAB * P, 2, P)
            .transpose(3, 0, 2, 1)
            .astype(_md.bfloat16))
    wcat = Ws.transpose(2, 0, 1).reshape(IN_FEAT, TD).astype(_md.bfloat16)
    a_r = As[:, OUT:, 0]  # [H, O]
    war = np.einsum("hof,ho->fh", Ws, a_r).astype(_md.bfloat16)
    iota = np.tile(np.arange(P, dtype=np.float32), (P, 1)).astype(_md.bfloat16)
    meta = cfg.meta_packed.astype(_md.bfloat16)
    in_maps = []
    for c in range(cfg.n_cores):
        in_maps.append({
            "xt": np.ascontiguousarray(xt), "wcat": wcat, "war": war,
            "iota": np.ascontiguousarray(iota),
            "idx": np.ascontiguousarray(cfg.idx_packed[c]),
            "meta": np.ascontiguousarray(meta[c]),
        })
    return in_maps


from concourse.bass_utils import run_bass_kernel_spmd

LAST_EXEC_TIME_NS = None
LAST_RESULTS = None


def kernel(x, src, dst, Ws, As):
    """Full-input entry point: shards internally across 8 NeuronCores."""
    global LAST_EXEC_TIME_NS, LAST_RESULTS
    x = np.asarray(x, np.float32)
    src = np.asarray(src)
    dst = np.asarray(dst)
    Ws = np.asarray(Ws, np.float32)
    As = np.asarray(As, np.float32)
    n = x.shape[0]

    cfg = Config(n, src, dst, n_cores=8)
    nc = build_program(cfg)
    in_maps = host_prep(cfg, x, Ws, As)
    import os as _os
    _trace = _os.environ.get("KERNEL_TRACE", "0") == "1"
    res = run_bass_kernel_spmd(nc, in_maps, core_ids=list(range(cfg.n_cores)),
                               trace=_trace)
    LAST_EXEC_TIME_NS = res.exec_time_ns
    LAST_RESULTS = res
    out = np.concatenate([res.results[c]["out"] for c in range(cfg.n_cores)],
                         axis=0)[:n]
    return np.ascontiguousarray(out, dtype=np.float32)
